# revision 2
# baseline (speedup 1.0000x reference)
"""ExpSyn kernel: diagonal linear recurrence isyn_t = beta*isyn_{t-1} + x_t.

Strategy:
  - Host: transpose data (B,T,N) -> (B,N,T) so time is contiguous per channel,
    and downcast to fp16 (tensor_tensor_scan carries its state in fp32
    regardless of operand dtype, so only the I/O is rounded; measured rel err
    ~5e-4 vs the 2e-2 gate). Halves HBM traffic vs fp32 -> ~2x on this
    memory-bound kernel.
  - Shard batch over 8 cores (2 batches/core -> 1024 rows of length T=4096).
  - Device: per 128-row block, 1MB fp16 DMA load (nc.sync ring), DVE
    tensor_tensor_scan (state = beta*state + x along the free/time dim,
    fp32 state, fp16 in/out), 1MB fp16 DMA store (nc.scalar ring — separate
    HWDGE FIFO so stores never head-of-line block loads). First block loads
    in geometric chunks (chained via initial=) so the DVE starts early;
    middle blocks scan/store in halves to smooth store bandwidth; last block
    stores in chunks so the tail is short.
  - Host: upcast fp16 -> fp32, gather, transpose back to (B,T,N).
"""

import numpy as np

DT = 1e-4
B, T, N = 16, 4096, 512
NCORES = 8
BLOC = B // NCORES          # 2 batches per core
ROWS = BLOC * N             # 1024 scan rows per core
NG = N // 128               # 4 channel groups of 128
NTILES = ROWS // 128        # 8 row-blocks per core

_cached = None


def _build():
    """Build + compile the single-core Bass program (run SPMD on 8 cores)."""
    import concourse.bacc as bacc
    import concourse.mybir as mybir
    from concourse import tile

    nc = bacc.Bacc("TRN2", debug=False, num_devices=NCORES)
    f32 = mybir.dt.float32
    f16 = mybir.dt.float16
    mult, add = mybir.AluOpType.mult, mybir.AluOpType.add

    x = nc.dram_tensor("x", [ROWS, T], f16, kind="ExternalInput")
    beta_d = nc.dram_tensor("beta", [128, NG], f32, kind="ExternalInput")
    y = nc.dram_tensor("y", [ROWS, T], f16, kind="ExternalOutput")

    with tile.TileContext(nc) as tc:
        with (
            tc.tile_pool(name="const", bufs=1) as cpool,
            tc.tile_pool(name="xin", bufs=8) as xpool,
        ):
            # tiny beta DMA rides the ACT ring (idle until the first store,
            # so it lands well before the first scan needs it)
            bsb = cpool.tile([128, NG], f32, name="bsb")
            nc.scalar.dma_start(out=bsb[:, :], in_=beta_d[:, :])

            def bcast(g, n):
                return bsb[:, g:g + 1].broadcast_to([128, n])

            # ---- block 0: chunked loads so the DVE starts ASAP ----
            # geometric chunk sizes: tiny first chunk -> earliest scan start
            bounds = [0, 128, 512, 1536, T]
            xt0 = xpool.tile([128, T], f16, tag="xt", name="xt0")
            for c in range(len(bounds) - 1):
                lo, hi = bounds[c], bounds[c + 1]
                nc.sync.dma_start(out=xt0[:, lo:hi], in_=x[0:128, lo:hi])
            for c in range(len(bounds) - 1):
                lo, hi = bounds[c], bounds[c + 1]
                init = 0.0 if c == 0 else xt0[:, lo - 1:lo]
                nc.vector.tensor_tensor_scan(
                    xt0[:, lo:hi], bcast(0, hi - lo), xt0[:, lo:hi],
                    init, mult, add)
            nc.scalar.dma_start(out=y[0:128, :], in_=xt0[:, :])

            # ---- blocks 1..6: 1MB load; scan + store in halves so the
            # store stream starts mid-scan and bandwidth stays smooth ----
            H = T // 2
            for k in range(1, NTILES - 1):
                g = k % NG
                xt = xpool.tile([128, T], f16, tag="xt", name=f"xt{k}")
                nc.sync.dma_start(out=xt[:, :], in_=x[k * 128:(k + 1) * 128, :])
                nc.vector.tensor_tensor_scan(
                    xt[:, 0:H], bcast(g, H), xt[:, 0:H], 0.0, mult, add)
                nc.scalar.dma_start(out=y[k * 128:(k + 1) * 128, 0:H],
                                    in_=xt[:, 0:H])
                nc.vector.tensor_tensor_scan(
                    xt[:, H:T], bcast(g, H), xt[:, H:T],
                    xt[:, H - 1:H], mult, add)
                nc.scalar.dma_start(out=y[k * 128:(k + 1) * 128, H:T],
                                    in_=xt[:, H:T])

            # ---- block 7: chunked stores so the tail is short ----
            k = NTILES - 1
            xt7 = xpool.tile([128, T], f16, tag="xt", name="xt7")
            nc.sync.dma_start(out=xt7[:, :], in_=x[k * 128:(k + 1) * 128, :])
            g = k % NG
            # shrinking chunks so the very last store is only 128KB
            bounds7 = [0, 1536, 2560, 3584, T]
            for c in range(len(bounds7) - 1):
                lo, hi = bounds7[c], bounds7[c + 1]
                init = 0.0 if c == 0 else xt7[:, lo - 1:lo]
                nc.vector.tensor_tensor_scan(
                    xt7[:, lo:hi], bcast(g, hi - lo), xt7[:, lo:hi],
                    init, mult, add)
                nc.scalar.dma_start(out=y[k * 128:(k + 1) * 128, lo:hi],
                                    in_=xt7[:, lo:hi])

    nc.compile()
    return nc


def _get_nc():
    global _cached
    if _cached is None:
        _cached = _build()
    return _cached


def _make_in_maps(data, tau_syn):
    tau = np.asarray(tau_syn, dtype=np.float64)
    beta = np.exp(-DT / tau).astype(np.float32)  # (1, N)
    beta_g = np.ascontiguousarray(beta.reshape(NG, 128).T)  # (128, NG)
    # (B, T, N) -> (B, N, T), batch-sharded across cores, fp16
    xt = np.asarray(data, dtype=np.float32).transpose(0, 2, 1).astype(np.float16)
    xt = np.ascontiguousarray(xt).reshape(NCORES, ROWS, T)
    return [{"x": xt[c], "beta": beta_g} for c in range(NCORES)]


def kernel(data, tau_syn):
    from concourse.bass_utils import run_bass_kernel_spmd

    nc = _get_nc()
    in_maps = _make_in_maps(data, tau_syn)
    res = run_bass_kernel_spmd(nc, in_maps, list(range(NCORES)))
    out = np.stack([res.results[c]["y"] for c in range(NCORES)])  # (8, ROWS, T)
    out = out.astype(np.float32).reshape(B, N, T).transpose(0, 2, 1)  # (B, T, N)
    return np.ascontiguousarray(out)
